# revision 18
# baseline (speedup 1.0000x reference)
"""GTN (Graph Transformer Network) message passing on 8 Trainium2 NeuronCores.

Problem nn_GTN_17162689314910:
  A: [E=5, N=2048, N] f32, X: [N, 256] f32, conv_w_*: [C=2, E, 1, 1] f32,
  gcn_weight: [256, 64] f32 -> out [N, C*64] f32.

Math per channel c (faithful to the reference):
  s_j = softmax(w_j[c]) over E for the three conv weight sets
  a = sum_e s0_e A_e ; b = sum_e s1_e A_e ; a1 = sum_e s2_e A_e
  H1 = a @ b
  H1n = colscale(H1, 1/(colsum_excl_diag + eps)), diag zeroed
  H2 = H1n @ a1
  H2' = H2 with diag set to 1 ; deg2 = colsum(H2') ; Xw = X @ W
  out[:, c*64:(c+1)*64] = relu(diag(1/(deg2+eps)) @ H2'.T @ Xw)

Sharding: channel c -> core group (cores 4c..4c+3); within a group row-blocks
of 512 rows. Each core streams the full A (bf16): TensorE builds b via
scaled-identity matmuls (PSUM-accumulated over E) while VectorE builds a1 and
a_shT with fused scalar_tensor_tensor chains. H1 is computed directly in
TRANSPOSED form (H1T = b_tiles.T @ a_shT) so no big on-device transposes are
needed. mm1'/mm2 run as fp8 DoubleRow matmuls (K=256/instr); the normalized
operand is pre-scaled by S=1024 to stay in fp8 range, compensated exactly in
the readout normalization. Only collectives: two [128,8] AllReduces for the
H1 column sums and a [2048,65] ReduceScatter for the readout partials, both
within the 4-core groups.
"""
import numpy as np
import ml_dtypes

import concourse.bacc as bacc
import concourse.mybir as mybir
import concourse.tile as tile
from concourse.bass_utils import run_bass_kernel_spmd

F32 = mybir.dt.float32
BF16 = mybir.dt.bfloat16
FP8 = mybir.dt.float8e4
U8 = mybir.dt.uint8
ALU = mybir.AluOpType
ACTF = mybir.ActivationFunctionType
DR = mybir.MatmulPerfMode.DoubleRow

E = 5
N = 2048
R = 512          # rows per core
C = 2
WOUT = 64
KT = N // 128    # 16 k/partition tiles
RT = R // 128    # 4 row tiles per core
EPS = 1e-8
S = 1024.0       # fp8 range scale for the normalized operand
GROUPS = [[0, 1, 2, 3], [4, 5, 6, 7]]
NCORES = 8

_BF = ml_dtypes.bfloat16
_F8 = ml_dtypes.float8_e4m3


def _build_nc():
    nc = bacc.Bacc("TRN2", target_bir_lowering=False, debug=False,
                   num_devices=NCORES)

    a_full = nc.dram_tensor("a_full", [E, N, N], BF16, kind="ExternalInput")
    a_rowst = nc.dram_tensor("a_rowst", [E, N, R], FP8, kind="ExternalInput")
    wbc = nc.dram_tensor("wbc", [128, 15], F32, kind="ExternalInput")
    eye_in = nc.dram_tensor("eye_in", [128, 128], BF16, kind="ExternalInput")
    mask1_in = nc.dram_tensor("mask1", [N, R], BF16, kind="ExternalInput")
    mask2_in = nc.dram_tensor("mask2", [R, N], U8, kind="ExternalInput")
    xt_loc = nc.dram_tensor("xt_loc", [256, R], BF16, kind="ExternalInput")
    w_gcn = nc.dram_tensor("w_gcn", [256, WOUT], BF16, kind="ExternalInput")
    out_loc = nc.dram_tensor("out_loc", [R, WOUT], F32, kind="ExternalOutput")

    with tile.TileContext(nc) as tc:
        with (
            tc.tile_pool(name="const", bufs=1) as constp,
            tc.tile_pool(name="big", bufs=1) as bigp,
            tc.tile_pool(name="stream", bufs=8) as streamp,
            tc.tile_pool(name="small", bufs=1) as smallp,
            tc.tile_pool(name="ps", bufs=8, space="PSUM") as psp,
            tc.tile_pool(name="dram", bufs=1, space="DRAM") as dramp,
        ):
            # ---- collective bounce buffers (internal DRAM)
            deg_in = dramp.tile([1, N], F32)
            deg_out = dramp.tile([1, N], F32)
            p_inA = dramp.tile([N // 2, WOUT + 1], F32)
            p_inB = dramp.tile([N // 2, WOUT + 1], F32)
            p_outA = dramp.tile([R // 2, WOUT + 1], F32)
            p_outB = dramp.tile([R // 2, WOUT + 1], F32)

            # ---- softmax of the three weight rows; scaled identities for b
            w_sb = constp.tile([128, 15], F32)
            nc.sync.dma_start(w_sb[:], wbc[:])
            eye_sb = constp.tile([128, 128], BF16)
            nc.sync.dma_start(eye_sb[:], eye_in[:])
            s_bc = constp.tile([128, 15], F32)
            for j in range(3):
                wj = w_sb[:, 5 * j:5 * j + 5]
                mx = constp.tile([128, 1], F32, tag=f"mx{j}")
                nc.vector.reduce_max(mx[:], wj, axis=mybir.AxisListType.X)
                t = constp.tile([128, 5], F32, tag=f"t{j}")
                nc.vector.tensor_scalar_sub(t[:], wj, mx[:])
                ex = constp.tile([128, 5], F32, tag=f"ex{j}")
                sm = constp.tile([128, 1], F32, tag=f"sm{j}")
                nc.scalar.activation(ex[:], t[:], ACTF.Exp, accum_out=sm[:])
                rc = constp.tile([128, 1], F32, tag=f"rc{j}")
                nc.vector.reciprocal(rc[:], sm[:])
                nc.vector.tensor_scalar_mul(s_bc[:, 5 * j:5 * j + 5], ex[:], rc[:])
            eyes = constp.tile([128, 10 * 128], BF16)  # s0/s1-scaled identities
            for j in range(2):
                for e in range(E):
                    i = 5 * j + e
                    nc.vector.tensor_scalar_mul(
                        eyes[:, 128 * i:128 * (i + 1)], eye_sb[:],
                        s_bc[:, i:i + 1])

            # ---- combine phase: stream full A (bf16)
            #   PE:  b = sum_e s1_e A_e        -> fp8 (via ScalarE copies)
            #   DVE: a1 = sum_e s2_e A_e       -> fp8 (stt chain)
            #   DVE: a_shT = sum_e s0_e A_e.T rows  -> fp8 (stt chain)
            b_sb = bigp.tile([128, KT * N], FP8, tag="slot_b")
            a1_sb = bigp.tile([128, KT * N], FP8, tag="slot_a1")
            a_shT = bigp.tile([128, KT * R], FP8, tag="slot_ashT")
            csa = constp.tile([128, KT], F32)   # rowsums of a_shT tiles
            csa_bf = constp.tile([128, KT], BF16)
            csa_rep = constp.tile([128, KT * 128], BF16)
            for kt in range(KT):
                ats = []
                for e in range(E):
                    at = streamp.tile([128, N], BF16, tag="afull", bufs=10,
                                      name=f"afull{kt}_{e}")
                    nc.sync.dma_start(at[:], a_full[e, 128 * kt:128 * (kt + 1), :])
                    ats.append(at)
                # PE: b tiles
                for nc4 in range(4):
                    sl = slice(512 * nc4, 512 * (nc4 + 1))
                    psb = psp.tile([128, 512], F32, tag="ps", name=f"bps{kt}_{nc4}")
                    for e in range(E):
                        nc.tensor.matmul(psb[:], eyes[:, 128 * (5 + e):128 * (6 + e)],
                                         ats[e][:, sl],
                                         start=(e == 0), stop=(e == E - 1))
                    nc.scalar.activation(
                        b_sb[:, N * kt + 512 * nc4:N * kt + 512 * (nc4 + 1)],
                        psb[:], ACTF.Copy)
                # DVE: a1 tile, ts (4x) + TT-add (2x) chain
                acc0 = streamp.tile([128, N], BF16, tag="acc0", bufs=1,
                                    name=f"acc0_{kt}")
                acc1 = streamp.tile([128, N], BF16, tag="acc1", bufs=1,
                                    name=f"acc1_{kt}")
                tmp = streamp.tile([128, N], BF16, tag="tmp", bufs=1,
                                   name=f"tmp_{kt}")
                nc.vector.tensor_scalar_mul(acc0[:], ats[0][:], s_bc[:, 10:11])
                nc.vector.tensor_scalar_mul(tmp[:], ats[1][:], s_bc[:, 11:12])
                nc.vector.tensor_add(acc1[:], tmp[:], acc0[:])
                nc.vector.tensor_scalar_mul(tmp[:], ats[2][:], s_bc[:, 12:13])
                nc.vector.tensor_add(acc0[:], tmp[:], acc1[:])
                nc.vector.tensor_scalar_mul(tmp[:], ats[3][:], s_bc[:, 13:14])
                nc.vector.tensor_add(acc1[:], tmp[:], acc0[:])
                nc.vector.tensor_scalar_mul(tmp[:], ats[4][:], s_bc[:, 14:15])
                nc.vector.tensor_add(a1_sb[:, N * kt:N * (kt + 1)], tmp[:],
                                     acc1[:])
                # PE: a_shT tile from fp8 transposed-rows stream
                psa = psp.tile([128, 512], F32, tag="ps", name=f"aps{kt}")
                for e in range(E):
                    art = streamp.tile([128, R], FP8, tag="art", bufs=10,
                                       name=f"art{kt}_{e}")
                    nc.sync.dma_start(art[:],
                                      a_rowst[e, 128 * kt:128 * (kt + 1), :])
                    nc.tensor.matmul(psa[:, :R],
                                     eyes[:, 128 * e:128 * (e + 1)], art[:],
                                     start=(e == 0), stop=(e == E - 1))
                nc.scalar.activation(a_shT[:, R * kt:R * (kt + 1)], psa[:, :R],
                                     ACTF.Copy, accum_out=csa[:, kt:kt + 1])
                nc.vector.tensor_copy(csa_bf[:, kt:kt + 1], csa[:, kt:kt + 1])
                nc.vector.tensor_copy(
                    csa_rep[:, 128 * kt:128 * (kt + 1)],
                    csa_bf[:, kt:kt + 1].broadcast_to([128, 128]))


            # ---- deg1 = csa . b  (column sums of H1 over local rows, incl
            # diag: a ~5e-4 relative effect on deg, far inside tolerance).
            # Fires the AllReduce before mm1' so the rendezvous hides there.
            degps = {}
            for nc4 in range(4):
                degps[nc4] = psp.tile([128, 512], F32, tag="ps",
                                      name=f"degps{nc4}")
            for kt in range(KT):
                for nc4 in range(4):
                    nc.tensor.matmul(
                        degps[nc4][:],
                        csa_rep[:, 128 * kt:128 * (kt + 1)],
                        b_sb[:, N * kt + 512 * nc4:N * kt + 512 * (nc4 + 1)],
                        start=(kt == 0), stop=(kt == KT - 1))
            degrow = smallp.tile([1, N], F32, tag="degrow")
            for nc4 in range(4):
                nc.vector.tensor_copy(degrow[0:1, 512 * nc4:512 * (nc4 + 1)],
                                      degps[nc4][0:1, :])
            nc.sync.dma_start(deg_in[:], degrow[:])
            nc.gpsimd.collective_compute(
                "AllReduce", ALU.add, replica_groups=GROUPS,
                ins=[deg_in[:].opt()], outs=[deg_out[:].opt()])
            degf = smallp.tile([128, KT], F32, tag="degf")
            nc.sync.dma_start(
                degf[:],
                deg_out[0:1, :].rearrange("o (k p) -> p (o k)", p=128))
            dege = smallp.tile([128, KT], F32, tag="dege")
            nc.vector.tensor_scalar_add(dege[:], degf[:], float(EPS))
            rcp = smallp.tile([128, KT], F32, tag="rcp")
            nc.vector.reciprocal(rcp[:], dege[:])
            dinv = smallp.tile([128, KT], F32, tag="dinv")
            nc.vector.tensor_scalar_mul(dinv[:], rcp[:], float(S))

            # ---- mm1': H1T = b.T @ a_shT  (fp8 DoubleRow, K=256/instr)
            h1tm = bigp.tile([128, KT * R], BF16, tag="slot_h1tm")
            bv = b_sb[:].rearrange("p (k n) -> p k n", k=KT)
            av = a_shT[:].rearrange("p (k m) -> p k m", k=KT)
            for half in range(2):
                nts = range(8 * half, 8 * half + 8)
                pss = {}
                for nt in nts:
                    pss[nt] = psp.tile([128, 512], F32, tag="ps", name=f"h1t_ps{nt}")
                for kp in range(KT // 2):
                    kt = 2 * kp
                    for nt in nts:
                        nc.tensor.matmul(
                            pss[nt][:, :R],
                            bv[:, kt:kt + 2, 128 * nt:128 * (nt + 1)],
                            av[:, kt:kt + 2, :],
                            start=(kp == 0), stop=(kp == KT // 2 - 1),
                            perf_mode=DR)
                for nt in nts:
                    nc.scalar.activation(h1tm[:, R * nt:R * (nt + 1)],
                                         pss[nt][:, :R], ACTF.Copy)

            # lhsT2 = (H1T masked) * S/(deg+eps)  -> fp8  (one fused op/tile)
            lhsT2 = bigp.tile([128, KT * R], FP8, tag="slot_lhsT2")
            for kt in range(KT):
                m1 = streamp.tile([128, R], BF16, tag="m1", bufs=4,
                                  name=f"m1_{kt}")
                nc.sync.dma_start(m1[:], mask1_in[128 * kt:128 * (kt + 1), :])
                nc.vector.scalar_tensor_tensor(
                    lhsT2[:, R * kt:R * (kt + 1)],
                    h1tm[:, R * kt:R * (kt + 1)], dinv[:, kt:kt + 1], m1[:],
                    op0=ALU.mult, op1=ALU.mult)

            # ---- mm2: S*H2 = lhsT2.T @ a1  (fp8 DoubleRow), diag := S
            h2 = bigp.tile([128, RT * N], BF16, tag="slot_h2")
            lv = lhsT2[:].rearrange("p (k m) -> p k m", k=KT)
            a1v = a1_sb[:].rearrange("p (k n) -> p k n", k=KT)
            ones_t = constp.tile([128, 1024], BF16)
            nc.vector.memset(ones_t[:], float(S))
            for half in range(2):
                ps2 = {}
                for mt in range(RT):
                    for nb in range(2):
                        ps2[(mt, nb)] = psp.tile([128, 512], F32, tag="ps",
                                                 name=f"h2_ps{mt}_{nb}")
                for kp in range(KT // 2):
                    kt = 2 * kp
                    noff = 1024 * half
                    for mt in range(RT):
                        lt = lv[:, kt:kt + 2, 128 * mt:128 * (mt + 1)]
                        for nb in range(2):
                            nc.tensor.matmul(
                                ps2[(mt, nb)],
                                lt,
                                a1v[:, kt:kt + 2,
                                    noff + 512 * nb:noff + 512 * (nb + 1)],
                                start=(kp == 0), stop=(kp == KT // 2 - 1),
                                perf_mode=DR)
                for mt in range(RT):
                    for nb in range(2):
                        dst = h2[:, N * mt + 1024 * half + 512 * nb:
                                 N * mt + 1024 * half + 512 * (nb + 1)]
                        if nb == 0:
                            nc.vector.tensor_copy(dst, ps2[(mt, nb)][:])
                        else:
                            nc.scalar.activation(dst, ps2[(mt, nb)][:], ACTF.Copy)
                for mt in range(RT):
                    m2 = streamp.tile([128, 1024], U8, tag="m2", bufs=2,
                                      name=f"m2_{mt}_{half}")
                    nc.sync.dma_start(
                        m2[:], mask2_in[128 * mt:128 * (mt + 1),
                                        1024 * half:1024 * (half + 1)])
                    nc.vector.copy_predicated(
                        h2[:, N * mt + 1024 * half:N * mt + 1024 * (half + 1)],
                        m2[:], ones_t[:])

            # ---- Xw = X @ W with ones column  [R, 65] bf16
            xw = constp.tile([128, RT * (WOUT + 1)], BF16)
            nc.vector.memset(xw[:], 1.0)
            xt_sb = constp.tile([128, 2 * R], BF16)
            nc.sync.dma_start(
                xt_sb[:].rearrange("p (f r) -> p f r", f=2),
                xt_loc[:].rearrange("(f p) r -> p f r", p=128))
            wg_sb = constp.tile([128, 2 * WOUT], BF16)
            nc.sync.dma_start(
                wg_sb[:].rearrange("p (f r) -> p f r", f=2),
                w_gcn[:].rearrange("(f p) r -> p f r", p=128))
            for jt in range(RT):
                psx = psp.tile([128, 512], F32, tag="ps", name=f"xw_ps{jt}")
                for ft in range(2):
                    nc.tensor.matmul(
                        psx[:, :WOUT],
                        xt_sb[:, R * ft + 128 * jt:R * ft + 128 * (jt + 1)],
                        wg_sb[:, WOUT * ft:WOUT * (ft + 1)],
                        start=(ft == 0), stop=(ft == 1))
                nc.vector.tensor_copy(
                    xw[:, (WOUT + 1) * jt:(WOUT + 1) * jt + WOUT], psx[:, :WOUT])

            # ---- mm3: P = (S H2').T @ Xw'  [N, 65] partials.
            # Two interleaved ReduceScatters so the first rendezvous overlaps
            # the second half of mm3 and the finalize of set A overlaps RS_B.
            # Set A = row-tiles it%4 in {0,1} -> out rows [512q, 512q+256);
            # set B = it%4 in {2,3} -> rows [512q+256, 512q+512).
            its_A = [it for it in range(KT) if it % 4 < 2]
            its_B = [it for it in range(KT) if it % 4 >= 2]
            for sidx, (its, pin) in enumerate([(its_A, p_inA), (its_B, p_inB)]):
                for it in its:
                    psp3 = psp.tile([128, 512], F32, tag="ps", name=f"p_ps{it}")
                    for jt in range(RT):
                        nc.tensor.matmul(
                            psp3[:, :WOUT + 1],
                            h2[:, N * jt + 128 * it:N * jt + 128 * (it + 1)],
                            xw[:, (WOUT + 1) * jt:(WOUT + 1) * (jt + 1)],
                            start=(jt == 0), stop=(jt == RT - 1))
                    row = 256 * (it // 4) + 128 * (it % 4 - 2 * sidx)
                    pt = streamp.tile([128, WOUT + 1], F32, tag="pt", bufs=4,
                                      name=f"pt{it}")
                    nc.vector.tensor_copy(pt[:], psp3[:, :WOUT + 1])
                    nc.sync.dma_start(pin[row:row + 128, :], pt[:])
                nc.gpsimd.collective_compute(
                    "ReduceScatter", ALU.add, replica_groups=GROUPS,
                    ins=[pin[:].opt()],
                    outs=[(p_outA if sidx == 0 else p_outB)[:].opt()])

            # ---- finalize: out = relu(P' / (deg2' + S*eps))   (S cancels)
            for sidx, pout in enumerate([p_outA, p_outB]):
                pf = smallp.tile([128, 2 * (WOUT + 1)], F32, tag=f"pf{sidx}",
                                 name=f"pf{sidx}")
                nc.sync.dma_start(
                    pf[:].rearrange("p (t f) -> p t f", t=2),
                    pout[:].rearrange("(t p) f -> p t f", p=128))
                of = smallp.tile([128, 2 * WOUT], F32, tag=f"of{sidx}",
                                 name=f"of{sidx}")
                for t in range(2):
                    d2 = smallp.tile([128, 1], F32, tag=f"d2_{sidx}_{t}",
                                     name=f"d2_{sidx}_{t}")
                    nc.vector.tensor_scalar_add(
                        d2[:], pf[:, (WOUT + 1) * t + WOUT:(WOUT + 1) * (t + 1)],
                        float(S * EPS))
                    r2 = smallp.tile([128, 1], F32, tag=f"r2_{sidx}_{t}",
                                     name=f"r2_{sidx}_{t}")
                    nc.vector.reciprocal(r2[:], d2[:])
                    nc.vector.tensor_scalar(
                        of[:, WOUT * t:WOUT * (t + 1)],
                        pf[:, (WOUT + 1) * t:(WOUT + 1) * t + WOUT],
                        r2[:], 0.0, op0=ALU.mult, op1=ALU.max)
                nc.sync.dma_start(
                    out_loc[256 * sidx:256 * (sidx + 1), :]
                        .rearrange("(t p) f -> p t f", p=128),
                    of[:].rearrange("p (t f) -> p t f", t=2))

    nc.compile()
    return nc


_NC_CACHE = []


def get_nc():
    if not _NC_CACHE:
        _NC_CACHE.append(_build_nc())
    return _NC_CACHE[0]


def make_in_maps(A, X, conv_w_l0_1, conv_w_l0_2, conv_w_l1, gcn_weight):
    A = np.asarray(A, np.float32)
    X = np.asarray(X, np.float32)
    w1 = np.asarray(conv_w_l0_1, np.float32)[:, :, 0, 0]
    w2 = np.asarray(conv_w_l0_2, np.float32)[:, :, 0, 0]
    w3 = np.asarray(conv_w_l1, np.float32)[:, :, 0, 0]
    W = np.asarray(gcn_weight, np.float32)

    a_bf = A.astype(_BF)
    eye = np.eye(128, dtype=_BF)
    wg = W.astype(_BF)

    in_maps = []
    for core in range(NCORES):
        c, q = core // 4, core % 4
        rows = slice(R * q, R * (q + 1))
        a_rowst = np.ascontiguousarray(
            A[:, rows, :].transpose(0, 2, 1)).astype(_F8)
        wrow = np.concatenate([w1[c], w2[c], w3[c]]).reshape(1, 15)
        wbc = np.ascontiguousarray(np.tile(wrow, (128, 1))).astype(np.float32)
        m1 = np.ones((N, R), dtype=_BF)
        g = np.arange(R)
        m1[R * q + g, g] = 0.0
        m2 = np.zeros((R, N), dtype=np.uint8)
        m2[g, R * q + g] = 1
        xt = np.ascontiguousarray(X[rows].T).astype(_BF)
        in_maps.append({
            "a_full": a_bf,
            "a_rowst": a_rowst,
            "wbc": wbc,
            "eye_in": eye,
            "mask1": m1,
            "mask2": m2,
            "xt_loc": xt,
            "w_gcn": wg,
        })
    return in_maps


def assemble(results):
    out = np.empty((N, C * WOUT), np.float32)
    for core in range(NCORES):
        c, q = core // 4, core % 4
        out[R * q:R * (q + 1), WOUT * c:WOUT * (c + 1)] = \
            results[core]["out_loc"]
    return out


def kernel(A, X, conv_w_l0_1, conv_w_l0_2, conv_w_l1, gcn_weight, **run_kwargs):
    nc = get_nc()
    in_maps = make_in_maps(A, X, conv_w_l0_1, conv_w_l0_2, conv_w_l1,
                           gcn_weight)
    res = run_bass_kernel_spmd(nc, in_maps, core_ids=list(range(NCORES)),
                               **run_kwargs)
    out = assemble(res.results)
    if run_kwargs:
        kernel.last_results = res
    return out


# revision 19
# speedup vs baseline: 1.0794x; 1.0794x over previous
"""GTN (Graph Transformer Network) message passing on 8 Trainium2 NeuronCores.

Problem nn_GTN_17162689314910:
  A: [E=5, N=2048, N] f32, X: [N, 256] f32, conv_w_*: [C=2, E, 1, 1] f32,
  gcn_weight: [256, 64] f32 -> out [N, C*64] f32.

Math per channel c (faithful to the reference):
  s_j = softmax(w_j[c]) over E for the three conv weight sets
  a = sum_e s0_e A_e ; b = sum_e s1_e A_e ; a1 = sum_e s2_e A_e
  H1 = a @ b
  H1n = colscale(H1, 1/(colsum_excl_diag + eps)), diag zeroed
  H2 = H1n @ a1
  H2' = H2 with diag set to 1 ; deg2 = colsum(H2') ; Xw = X @ W
  out[:, c*64:(c+1)*64] = relu(diag(1/(deg2+eps)) @ H2'.T @ Xw)

Sharding: channel c -> core group (cores 4c..4c+3); within a group row-blocks
of 512 rows. Each core streams the full A (bf16): TensorE builds b via
scaled-identity matmuls (PSUM-accumulated over E) while VectorE builds a1 and
a_shT with fused scalar_tensor_tensor chains. H1 is computed directly in
TRANSPOSED form (H1T = b_tiles.T @ a_shT) so no big on-device transposes are
needed. mm1'/mm2 run as fp8 DoubleRow matmuls (K=256/instr); the normalized
operand is pre-scaled by S=1024 to stay in fp8 range, compensated exactly in
the readout normalization. Only collectives: two [128,8] AllReduces for the
H1 column sums and a [2048,65] ReduceScatter for the readout partials, both
within the 4-core groups.
"""
import numpy as np
import ml_dtypes

import concourse.bacc as bacc
import concourse.mybir as mybir
import concourse.tile as tile
from concourse.bass_utils import run_bass_kernel_spmd

F32 = mybir.dt.float32
BF16 = mybir.dt.bfloat16
FP8 = mybir.dt.float8e4
U8 = mybir.dt.uint8
ALU = mybir.AluOpType
ACTF = mybir.ActivationFunctionType
DR = mybir.MatmulPerfMode.DoubleRow

E = 5
N = 2048
R = 512          # rows per core
C = 2
WOUT = 64
KT = N // 128    # 16 k/partition tiles
RT = R // 128    # 4 row tiles per core
EPS = 1e-8
S = 1024.0       # fp8 range scale for the normalized operand
GROUPS = [[0, 1, 2, 3], [4, 5, 6, 7]]
NCORES = 8

_BF = ml_dtypes.bfloat16
_F8 = ml_dtypes.float8_e4m3


def _build_nc():
    nc = bacc.Bacc("TRN2", target_bir_lowering=False, debug=False,
                   num_devices=NCORES)

    a_full = nc.dram_tensor("a_full", [E, N, N], BF16, kind="ExternalInput")
    a_rowst = nc.dram_tensor("a_rowst", [E, N, R], FP8, kind="ExternalInput")
    wbc = nc.dram_tensor("wbc", [128, 15], F32, kind="ExternalInput")
    eye_in = nc.dram_tensor("eye_in", [128, 128], BF16, kind="ExternalInput")
    mask1_in = nc.dram_tensor("mask1", [N, R], BF16, kind="ExternalInput")
    mask2_in = nc.dram_tensor("mask2", [R, N], U8, kind="ExternalInput")
    xt_loc = nc.dram_tensor("xt_loc", [256, R], BF16, kind="ExternalInput")
    w_gcn = nc.dram_tensor("w_gcn", [256, WOUT], BF16, kind="ExternalInput")
    out_loc = nc.dram_tensor("out_loc", [R, WOUT], F32, kind="ExternalOutput")

    with tile.TileContext(nc) as tc:
        with (
            tc.tile_pool(name="const", bufs=1) as constp,
            tc.tile_pool(name="big", bufs=1) as bigp,
            tc.tile_pool(name="stream", bufs=8) as streamp,
            tc.tile_pool(name="small", bufs=1) as smallp,
            tc.tile_pool(name="ps", bufs=8, space="PSUM") as psp,
            tc.tile_pool(name="dram", bufs=1, space="DRAM") as dramp,
        ):
            # ---- collective bounce buffers (internal DRAM)
            deg_in = dramp.tile([1, N], F32)
            deg_out = dramp.tile([1, N], F32)
            p_inA = dramp.tile([N // 2, WOUT + 1], F32)
            p_inB = dramp.tile([N // 2, WOUT + 1], F32)
            p_outA = dramp.tile([R // 2, WOUT + 1], F32)
            p_outB = dramp.tile([R // 2, WOUT + 1], F32)

            # ---- softmax of the three weight rows; scaled identities for b
            w_sb = constp.tile([128, 15], F32)
            nc.sync.dma_start(w_sb[:], wbc[:])
            eye_sb = constp.tile([128, 128], BF16)
            nc.sync.dma_start(eye_sb[:], eye_in[:])
            s_bc = constp.tile([128, 15], F32)
            for j in range(3):
                wj = w_sb[:, 5 * j:5 * j + 5]
                mx = constp.tile([128, 1], F32, tag=f"mx{j}")
                nc.vector.reduce_max(mx[:], wj, axis=mybir.AxisListType.X)
                t = constp.tile([128, 5], F32, tag=f"t{j}")
                nc.vector.tensor_scalar_sub(t[:], wj, mx[:])
                ex = constp.tile([128, 5], F32, tag=f"ex{j}")
                sm = constp.tile([128, 1], F32, tag=f"sm{j}")
                nc.scalar.activation(ex[:], t[:], ACTF.Exp, accum_out=sm[:])
                rc = constp.tile([128, 1], F32, tag=f"rc{j}")
                nc.vector.reciprocal(rc[:], sm[:])
                nc.vector.tensor_scalar_mul(s_bc[:, 5 * j:5 * j + 5], ex[:], rc[:])
            eyes = constp.tile([128, 10 * 128], BF16)  # s0/s1-scaled identities
            for j in range(2):
                for e in range(E):
                    i = 5 * j + e
                    nc.vector.tensor_scalar_mul(
                        eyes[:, 128 * i:128 * (i + 1)], eye_sb[:],
                        s_bc[:, i:i + 1])

            # ---- combine phase: stream full A (bf16)
            #   PE:  b = sum_e s1_e A_e        -> fp8 (via ScalarE copies)
            #   DVE: a1 = sum_e s2_e A_e       -> fp8 (stt chain)
            #   DVE: a_shT = sum_e s0_e A_e.T rows  -> fp8 (stt chain)
            b_sb = bigp.tile([128, KT * N], FP8, tag="slot_b")
            a1_sb = bigp.tile([128, KT * N], FP8, tag="slot_a1")
            a_shT = bigp.tile([128, KT * R], FP8, tag="slot_ashT")
            csa = constp.tile([128, KT], F32)   # rowsums of a_shT tiles
            csa_bf = constp.tile([128, KT], BF16)
            csa_rep = constp.tile([128, KT * 128], BF16)
            degps = {}
            for nc4 in range(4):
                degps[nc4] = psp.tile([128, 512], F32, tag="ps",
                                      name=f"degps{nc4}")
            for kt in range(KT):
                ats = []
                for e in range(E):
                    at = streamp.tile([128, N], BF16, tag="afull", bufs=10,
                                      name=f"afull{kt}_{e}")
                    nc.sync.dma_start(at[:], a_full[e, 128 * kt:128 * (kt + 1), :])
                    ats.append(at)
                # PE: b tiles
                for nc4 in range(4):
                    sl = slice(512 * nc4, 512 * (nc4 + 1))
                    psb = psp.tile([128, 512], F32, tag="ps", name=f"bps{kt}_{nc4}")
                    for e in range(E):
                        nc.tensor.matmul(psb[:], eyes[:, 128 * (5 + e):128 * (6 + e)],
                                         ats[e][:, sl],
                                         start=(e == 0), stop=(e == E - 1))
                    nc.scalar.activation(
                        b_sb[:, N * kt + 512 * nc4:N * kt + 512 * (nc4 + 1)],
                        psb[:], ACTF.Copy)
                # DVE: a1 tile, ts (4x) + TT-add (2x) chain
                acc0 = streamp.tile([128, N], BF16, tag="acc0", bufs=1,
                                    name=f"acc0_{kt}")
                acc1 = streamp.tile([128, N], BF16, tag="acc1", bufs=1,
                                    name=f"acc1_{kt}")
                tmp = streamp.tile([128, N], BF16, tag="tmp", bufs=1,
                                   name=f"tmp_{kt}")
                t1 = streamp.tile([128, N], BF16, tag="t1", bufs=1,
                                  name=f"t1_{kt}")
                t3 = streamp.tile([128, N], BF16, tag="t3", bufs=1,
                                  name=f"t3_{kt}")
                nc.scalar.activation(t1[:], ats[1][:], ACTF.Copy,
                                     scale=s_bc[:, 11:12])
                nc.scalar.activation(t3[:], ats[3][:], ACTF.Copy,
                                     scale=s_bc[:, 13:14])
                nc.vector.tensor_scalar_mul(acc0[:], ats[0][:], s_bc[:, 10:11])
                nc.vector.tensor_add(acc1[:], t1[:], acc0[:])
                nc.vector.tensor_scalar_mul(tmp[:], ats[2][:], s_bc[:, 12:13])
                nc.vector.tensor_add(acc0[:], tmp[:], acc1[:])
                nc.vector.tensor_add(acc1[:], t3[:], acc0[:])
                nc.vector.tensor_scalar_mul(tmp[:], ats[4][:], s_bc[:, 14:15])
                nc.vector.tensor_add(a1_sb[:, N * kt:N * (kt + 1)], tmp[:],
                                     acc1[:])
                # PE: a_shT tile from fp8 transposed-rows stream
                psa = psp.tile([128, 512], F32, tag="ps", name=f"aps{kt}")
                for e in range(E):
                    art = streamp.tile([128, R], FP8, tag="art", bufs=10,
                                       name=f"art{kt}_{e}")
                    nc.sync.dma_start(art[:],
                                      a_rowst[e, 128 * kt:128 * (kt + 1), :])
                    nc.tensor.matmul(psa[:, :R],
                                     eyes[:, 128 * e:128 * (e + 1)], art[:],
                                     start=(e == 0), stop=(e == E - 1))
                nc.scalar.activation(a_shT[:, R * kt:R * (kt + 1)], psa[:, :R],
                                     ACTF.Copy, accum_out=csa[:, kt:kt + 1])
                nc.vector.tensor_copy(csa_bf[:, kt:kt + 1], csa[:, kt:kt + 1])
                nc.vector.tensor_copy(
                    csa_rep[:, 128 * kt:128 * (kt + 1)],
                    csa_bf[:, kt:kt + 1].broadcast_to([128, 128]))
                for nc4 in range(4):
                    nc.tensor.matmul(
                        degps[nc4][:],
                        csa_rep[:, 128 * kt:128 * (kt + 1)],
                        b_sb[:, N * kt + 512 * nc4:N * kt + 512 * (nc4 + 1)],
                        start=(kt == 0), stop=(kt == KT - 1))


            # ---- deg1 = csa . b  (column sums of H1 over local rows, incl
            # diag: a ~5e-4 relative effect on deg, far inside tolerance).
            # Fires the AllReduce before mm1' so the rendezvous hides there.
            degrow = smallp.tile([1, N], F32, tag="degrow")
            for nc4 in range(4):
                nc.vector.tensor_copy(degrow[0:1, 512 * nc4:512 * (nc4 + 1)],
                                      degps[nc4][0:1, :])
            nc.sync.dma_start(deg_in[:], degrow[:])
            nc.gpsimd.collective_compute(
                "AllReduce", ALU.add, replica_groups=GROUPS,
                ins=[deg_in[:].opt()], outs=[deg_out[:].opt()])
            degf = smallp.tile([128, KT], F32, tag="degf")
            nc.sync.dma_start(
                degf[:],
                deg_out[0:1, :].rearrange("o (k p) -> p (o k)", p=128))
            dege = smallp.tile([128, KT], F32, tag="dege")
            nc.vector.tensor_scalar_add(dege[:], degf[:], float(EPS))
            rcp = smallp.tile([128, KT], F32, tag="rcp")
            nc.vector.reciprocal(rcp[:], dege[:])
            dinv = smallp.tile([128, KT], F32, tag="dinv")
            nc.vector.tensor_scalar_mul(dinv[:], rcp[:], float(S))

            # ---- mm1': H1T = b.T @ a_shT  (fp8 DoubleRow, K=256/instr)
            h1tm = bigp.tile([128, KT * R], BF16, tag="slot_h1tm")
            bv = b_sb[:].rearrange("p (k n) -> p k n", k=KT)
            av = a_shT[:].rearrange("p (k m) -> p k m", k=KT)
            for half in range(2):
                nts = range(8 * half, 8 * half + 8)
                pss = {}
                for nt in nts:
                    pss[nt] = psp.tile([128, 512], F32, tag="ps", name=f"h1t_ps{nt}")
                for kp in range(KT // 2):
                    kt = 2 * kp
                    for nt in nts:
                        nc.tensor.matmul(
                            pss[nt][:, :R],
                            bv[:, kt:kt + 2, 128 * nt:128 * (nt + 1)],
                            av[:, kt:kt + 2, :],
                            start=(kp == 0), stop=(kp == KT // 2 - 1),
                            perf_mode=DR)
                for nt in nts:
                    nc.scalar.activation(h1tm[:, R * nt:R * (nt + 1)],
                                         pss[nt][:, :R], ACTF.Copy)

            # lhsT2 = (H1T masked) * S/(deg+eps)  -> fp8  (one fused op/tile)
            lhsT2 = bigp.tile([128, KT * R], FP8, tag="slot_lhsT2")
            for kt in range(KT):
                m1 = streamp.tile([128, R], BF16, tag="m1", bufs=4,
                                  name=f"m1_{kt}")
                nc.sync.dma_start(m1[:], mask1_in[128 * kt:128 * (kt + 1), :])
                nc.vector.scalar_tensor_tensor(
                    lhsT2[:, R * kt:R * (kt + 1)],
                    h1tm[:, R * kt:R * (kt + 1)], dinv[:, kt:kt + 1], m1[:],
                    op0=ALU.mult, op1=ALU.mult)

            # ---- mm2: S*H2 = lhsT2.T @ a1  (fp8 DoubleRow), diag := S
            h2 = bigp.tile([128, RT * N], BF16, tag="slot_h2")
            lv = lhsT2[:].rearrange("p (k m) -> p k m", k=KT)
            a1v = a1_sb[:].rearrange("p (k n) -> p k n", k=KT)
            ones_t = constp.tile([128, 1024], BF16)
            nc.vector.memset(ones_t[:], float(S))
            for half in range(2):
                ps2 = {}
                for mt in range(RT):
                    for nb in range(2):
                        ps2[(mt, nb)] = psp.tile([128, 512], F32, tag="ps",
                                                 name=f"h2_ps{mt}_{nb}")
                for kp in range(KT // 2):
                    kt = 2 * kp
                    noff = 1024 * half
                    for mt in range(RT):
                        lt = lv[:, kt:kt + 2, 128 * mt:128 * (mt + 1)]
                        for nb in range(2):
                            nc.tensor.matmul(
                                ps2[(mt, nb)],
                                lt,
                                a1v[:, kt:kt + 2,
                                    noff + 512 * nb:noff + 512 * (nb + 1)],
                                start=(kp == 0), stop=(kp == KT // 2 - 1),
                                perf_mode=DR)
                for mt in range(RT):
                    for nb in range(2):
                        dst = h2[:, N * mt + 1024 * half + 512 * nb:
                                 N * mt + 1024 * half + 512 * (nb + 1)]
                        if nb == 0:
                            nc.vector.tensor_copy(dst, ps2[(mt, nb)][:])
                        else:
                            nc.scalar.activation(dst, ps2[(mt, nb)][:], ACTF.Copy)
                for mt in range(RT):
                    m2 = streamp.tile([128, 1024], U8, tag="m2", bufs=2,
                                      name=f"m2_{mt}_{half}")
                    nc.sync.dma_start(
                        m2[:], mask2_in[128 * mt:128 * (mt + 1),
                                        1024 * half:1024 * (half + 1)])
                    nc.vector.copy_predicated(
                        h2[:, N * mt + 1024 * half:N * mt + 1024 * (half + 1)],
                        m2[:], ones_t[:])

            # ---- Xw = X @ W with ones column  [R, 65] bf16
            xw = constp.tile([128, RT * (WOUT + 1)], BF16)
            nc.vector.memset(xw[:], 1.0)
            xt_sb = constp.tile([128, 2 * R], BF16)
            nc.sync.dma_start(
                xt_sb[:].rearrange("p (f r) -> p f r", f=2),
                xt_loc[:].rearrange("(f p) r -> p f r", p=128))
            wg_sb = constp.tile([128, 2 * WOUT], BF16)
            nc.sync.dma_start(
                wg_sb[:].rearrange("p (f r) -> p f r", f=2),
                w_gcn[:].rearrange("(f p) r -> p f r", p=128))
            for jt in range(RT):
                psx = psp.tile([128, 512], F32, tag="ps", name=f"xw_ps{jt}")
                for ft in range(2):
                    nc.tensor.matmul(
                        psx[:, :WOUT],
                        xt_sb[:, R * ft + 128 * jt:R * ft + 128 * (jt + 1)],
                        wg_sb[:, WOUT * ft:WOUT * (ft + 1)],
                        start=(ft == 0), stop=(ft == 1))
                nc.vector.tensor_copy(
                    xw[:, (WOUT + 1) * jt:(WOUT + 1) * jt + WOUT], psx[:, :WOUT])

            # ---- mm3: P = (S H2').T @ Xw'  [N, 65] partials.
            # Two interleaved ReduceScatters so the first rendezvous overlaps
            # the second half of mm3 and the finalize of set A overlaps RS_B.
            # Set A = row-tiles it%4 in {0,1} -> out rows [512q, 512q+256);
            # set B = it%4 in {2,3} -> rows [512q+256, 512q+512).
            its_A = [it for it in range(KT) if it % 4 < 2]
            its_B = [it for it in range(KT) if it % 4 >= 2]
            for sidx, (its, pin) in enumerate([(its_A, p_inA), (its_B, p_inB)]):
                for it in its:
                    psp3 = psp.tile([128, 512], F32, tag="ps", name=f"p_ps{it}")
                    for jt in range(RT):
                        nc.tensor.matmul(
                            psp3[:, :WOUT + 1],
                            h2[:, N * jt + 128 * it:N * jt + 128 * (it + 1)],
                            xw[:, (WOUT + 1) * jt:(WOUT + 1) * (jt + 1)],
                            start=(jt == 0), stop=(jt == RT - 1))
                    row = 256 * (it // 4) + 128 * (it % 4 - 2 * sidx)
                    pt = streamp.tile([128, WOUT + 1], F32, tag="pt", bufs=4,
                                      name=f"pt{it}")
                    nc.vector.tensor_copy(pt[:], psp3[:, :WOUT + 1])
                    nc.sync.dma_start(pin[row:row + 128, :], pt[:])
                nc.gpsimd.collective_compute(
                    "ReduceScatter", ALU.add, replica_groups=GROUPS,
                    ins=[pin[:].opt()],
                    outs=[(p_outA if sidx == 0 else p_outB)[:].opt()])

            # ---- finalize: out = relu(P' / (deg2' + S*eps))   (S cancels)
            for sidx, pout in enumerate([p_outA, p_outB]):
                pf = smallp.tile([128, 2 * (WOUT + 1)], F32, tag=f"pf{sidx}",
                                 name=f"pf{sidx}")
                nc.sync.dma_start(
                    pf[:].rearrange("p (t f) -> p t f", t=2),
                    pout[:].rearrange("(t p) f -> p t f", p=128))
                of = smallp.tile([128, 2 * WOUT], F32, tag=f"of{sidx}",
                                 name=f"of{sidx}")
                for t in range(2):
                    d2 = smallp.tile([128, 1], F32, tag=f"d2_{sidx}_{t}",
                                     name=f"d2_{sidx}_{t}")
                    nc.vector.tensor_scalar_add(
                        d2[:], pf[:, (WOUT + 1) * t + WOUT:(WOUT + 1) * (t + 1)],
                        float(S * EPS))
                    r2 = smallp.tile([128, 1], F32, tag=f"r2_{sidx}_{t}",
                                     name=f"r2_{sidx}_{t}")
                    nc.vector.reciprocal(r2[:], d2[:])
                    nc.vector.tensor_scalar(
                        of[:, WOUT * t:WOUT * (t + 1)],
                        pf[:, (WOUT + 1) * t:(WOUT + 1) * t + WOUT],
                        r2[:], 0.0, op0=ALU.mult, op1=ALU.max)
                nc.sync.dma_start(
                    out_loc[256 * sidx:256 * (sidx + 1), :]
                        .rearrange("(t p) f -> p t f", p=128),
                    of[:].rearrange("p (t f) -> p t f", t=2))

    nc.compile()
    return nc


_NC_CACHE = []


def get_nc():
    if not _NC_CACHE:
        _NC_CACHE.append(_build_nc())
    return _NC_CACHE[0]


def make_in_maps(A, X, conv_w_l0_1, conv_w_l0_2, conv_w_l1, gcn_weight):
    A = np.asarray(A, np.float32)
    X = np.asarray(X, np.float32)
    w1 = np.asarray(conv_w_l0_1, np.float32)[:, :, 0, 0]
    w2 = np.asarray(conv_w_l0_2, np.float32)[:, :, 0, 0]
    w3 = np.asarray(conv_w_l1, np.float32)[:, :, 0, 0]
    W = np.asarray(gcn_weight, np.float32)

    a_bf = A.astype(_BF)
    eye = np.eye(128, dtype=_BF)
    wg = W.astype(_BF)

    in_maps = []
    for core in range(NCORES):
        c, q = core // 4, core % 4
        rows = slice(R * q, R * (q + 1))
        a_rowst = np.ascontiguousarray(
            A[:, rows, :].transpose(0, 2, 1)).astype(_F8)
        wrow = np.concatenate([w1[c], w2[c], w3[c]]).reshape(1, 15)
        wbc = np.ascontiguousarray(np.tile(wrow, (128, 1))).astype(np.float32)
        m1 = np.ones((N, R), dtype=_BF)
        g = np.arange(R)
        m1[R * q + g, g] = 0.0
        m2 = np.zeros((R, N), dtype=np.uint8)
        m2[g, R * q + g] = 1
        xt = np.ascontiguousarray(X[rows].T).astype(_BF)
        in_maps.append({
            "a_full": a_bf,
            "a_rowst": a_rowst,
            "wbc": wbc,
            "eye_in": eye,
            "mask1": m1,
            "mask2": m2,
            "xt_loc": xt,
            "w_gcn": wg,
        })
    return in_maps


def assemble(results):
    out = np.empty((N, C * WOUT), np.float32)
    for core in range(NCORES):
        c, q = core // 4, core % 4
        out[R * q:R * (q + 1), WOUT * c:WOUT * (c + 1)] = \
            results[core]["out_loc"]
    return out


def kernel(A, X, conv_w_l0_1, conv_w_l0_2, conv_w_l1, gcn_weight, **run_kwargs):
    nc = get_nc()
    in_maps = make_in_maps(A, X, conv_w_l0_1, conv_w_l0_2, conv_w_l1,
                           gcn_weight)
    res = run_bass_kernel_spmd(nc, in_maps, core_ids=list(range(NCORES)),
                               **run_kwargs)
    out = assemble(res.results)
    if run_kwargs:
        kernel.last_results = res
    return out


# revision 20
# speedup vs baseline: 1.1367x; 1.0531x over previous
"""GTN (Graph Transformer Network) message passing on 8 Trainium2 NeuronCores.

Problem nn_GTN_17162689314910:
  A: [E=5, N=2048, N] f32, X: [N, 256] f32, conv_w_*: [C=2, E, 1, 1] f32,
  gcn_weight: [256, 64] f32 -> out [N, C*64] f32.

Math per channel c (faithful to the reference):
  s_j = softmax(w_j[c]) over E for the three conv weight sets
  a = sum_e s0_e A_e ; b = sum_e s1_e A_e ; a1 = sum_e s2_e A_e
  H1 = a @ b
  H1n = colscale(H1, 1/(colsum_excl_diag + eps)), diag zeroed
  H2 = H1n @ a1
  H2' = H2 with diag set to 1 ; deg2 = colsum(H2') ; Xw = X @ W
  out[:, c*64:(c+1)*64] = relu(diag(1/(deg2+eps)) @ H2'.T @ Xw)

Sharding: channel c -> core group (cores 4c..4c+3); within a group row-blocks
of 512 rows. Each core streams the full A (bf16): TensorE builds b via
scaled-identity matmuls (PSUM-accumulated over E) while VectorE builds a1 and
a_shT with fused scalar_tensor_tensor chains. H1 is computed directly in
TRANSPOSED form (H1T = b_tiles.T @ a_shT) so no big on-device transposes are
needed. mm1'/mm2 run as fp8 DoubleRow matmuls (K=256/instr); the normalized
operand is pre-scaled by S=1024 to stay in fp8 range, compensated exactly in
the readout normalization. The H1 column sums are computed WITHOUT a second pass
over H1: deg1_local = csa . b, where csa (column sums of the local a-block)
falls out of the a_shT PSUM-copy via accum_out; the [1,2048] AllReduce fires
at combine end and hides under mm1'. Readout partials go through two
interleaved [1024,65] ReduceScatters. All collectives are within the 4-core
groups.
"""
import numpy as np
import ml_dtypes

import concourse.bacc as bacc
import concourse.mybir as mybir
import concourse.tile as tile
from concourse.bass_utils import run_bass_kernel_spmd

F32 = mybir.dt.float32
BF16 = mybir.dt.bfloat16
FP8 = mybir.dt.float8e4
U8 = mybir.dt.uint8
ALU = mybir.AluOpType
ACTF = mybir.ActivationFunctionType
DR = mybir.MatmulPerfMode.DoubleRow

E = 5
N = 2048
R = 512          # rows per core
C = 2
WOUT = 64
KT = N // 128    # 16 k/partition tiles
RT = R // 128    # 4 row tiles per core
EPS = 1e-8
S = 1024.0       # fp8 range scale for the normalized operand
GROUPS = [[0, 1, 2, 3], [4, 5, 6, 7]]
NCORES = 8

_BF = ml_dtypes.bfloat16
_F8 = ml_dtypes.float8_e4m3


def _build_nc():
    nc = bacc.Bacc("TRN2", target_bir_lowering=False, debug=False,
                   num_devices=NCORES)

    a_full = nc.dram_tensor("a_full", [E, N, N], BF16, kind="ExternalInput")
    a_rowst = nc.dram_tensor("a_rowst", [E, N, R], FP8, kind="ExternalInput")
    wbc = nc.dram_tensor("wbc", [128, 15], F32, kind="ExternalInput")
    eye_in = nc.dram_tensor("eye_in", [128, 128], BF16, kind="ExternalInput")
    mask1_in = nc.dram_tensor("mask1", [N, R], BF16, kind="ExternalInput")
    mask2_in = nc.dram_tensor("mask2", [R, N], U8, kind="ExternalInput")
    xt_loc = nc.dram_tensor("xt_loc", [256, R], BF16, kind="ExternalInput")
    w_gcn = nc.dram_tensor("w_gcn", [256, WOUT], BF16, kind="ExternalInput")
    out_loc = nc.dram_tensor("out_loc", [R, WOUT], F32, kind="ExternalOutput")

    with tile.TileContext(nc) as tc:
        with (
            tc.tile_pool(name="const", bufs=1) as constp,
            tc.tile_pool(name="big", bufs=1) as bigp,
            tc.tile_pool(name="stream", bufs=8) as streamp,
            tc.tile_pool(name="small", bufs=1) as smallp,
            tc.tile_pool(name="ps", bufs=8, space="PSUM") as psp,
            tc.tile_pool(name="dram", bufs=1, space="DRAM") as dramp,
        ):
            # ---- collective bounce buffers (internal DRAM)
            deg_in = dramp.tile([1, N], F32)
            deg_out = dramp.tile([1, N], F32)
            p_inA = dramp.tile([N // 2, WOUT + 1], F32)
            p_inB = dramp.tile([N // 2, WOUT + 1], F32)
            p_outA = dramp.tile([R // 2, WOUT + 1], F32)
            p_outB = dramp.tile([R // 2, WOUT + 1], F32)

            # ---- softmax of the three weight rows; scaled identities for b
            w_sb = constp.tile([128, 15], F32)
            nc.sync.dma_start(w_sb[:], wbc[:])
            eye_sb = constp.tile([128, 128], BF16)
            nc.sync.dma_start(eye_sb[:], eye_in[:])
            s_bc = constp.tile([128, 15], F32)
            for j in range(3):
                wj = w_sb[:, 5 * j:5 * j + 5]
                mx = constp.tile([128, 1], F32, tag=f"mx{j}")
                nc.vector.reduce_max(mx[:], wj, axis=mybir.AxisListType.X)
                t = constp.tile([128, 5], F32, tag=f"t{j}")
                nc.vector.tensor_scalar_sub(t[:], wj, mx[:])
                ex = constp.tile([128, 5], F32, tag=f"ex{j}")
                sm = constp.tile([128, 1], F32, tag=f"sm{j}")
                nc.scalar.activation(ex[:], t[:], ACTF.Exp, accum_out=sm[:])
                rc = constp.tile([128, 1], F32, tag=f"rc{j}")
                nc.vector.reciprocal(rc[:], sm[:])
                nc.vector.tensor_scalar_mul(s_bc[:, 5 * j:5 * j + 5], ex[:], rc[:])
            eyes = constp.tile([128, 10 * 128], BF16)  # s0/s1-scaled identities
            for j in range(2):
                for e in range(E):
                    i = 5 * j + e
                    nc.vector.tensor_scalar_mul(
                        eyes[:, 128 * i:128 * (i + 1)], eye_sb[:],
                        s_bc[:, i:i + 1])

            # ---- combine phase: stream full A (bf16)
            #   PE:  b = sum_e s1_e A_e        -> fp8 (via ScalarE copies)
            #   DVE: a1 = sum_e s2_e A_e       -> fp8 (stt chain)
            #   DVE: a_shT = sum_e s0_e A_e.T rows  -> fp8 (stt chain)
            b_sb = bigp.tile([128, KT * N], FP8, tag="slot_b")
            a1_sb = bigp.tile([128, KT * N], FP8, tag="slot_a1")
            a_shT = bigp.tile([128, KT * R], FP8, tag="slot_ashT")
            csa = constp.tile([128, KT], F32)   # rowsums of a_shT tiles
            csa_bf = constp.tile([128, KT], BF16)
            csa_rep = constp.tile([128, KT * 128], BF16)
            degps = {}
            for nc4 in range(4):
                degps[nc4] = psp.tile([128, 512], F32, tag="ps",
                                      name=f"degps{nc4}")
            for kt in range(KT):
                ats = []
                for e in range(E):
                    at = streamp.tile([128, N], BF16, tag="afull", bufs=10,
                                      name=f"afull{kt}_{e}")
                    nc.sync.dma_start(at[:], a_full[e, 128 * kt:128 * (kt + 1), :])
                    ats.append(at)
                # PE: b tiles
                for nc4 in range(4):
                    sl = slice(512 * nc4, 512 * (nc4 + 1))
                    psb = psp.tile([128, 512], F32, tag="ps", name=f"bps{kt}_{nc4}")
                    for e in range(E):
                        nc.tensor.matmul(psb[:], eyes[:, 128 * (5 + e):128 * (6 + e)],
                                         ats[e][:, sl],
                                         start=(e == 0), stop=(e == E - 1))
                    nc.scalar.activation(
                        b_sb[:, N * kt + 512 * nc4:N * kt + 512 * (nc4 + 1)],
                        psb[:], ACTF.Copy)
                # DVE: a1 tile, ts (4x) + TT-add (2x) chain
                acc0 = streamp.tile([128, N], BF16, tag="acc0", bufs=1,
                                    name=f"acc0_{kt}")
                acc1 = streamp.tile([128, N], BF16, tag="acc1", bufs=1,
                                    name=f"acc1_{kt}")
                tmp = streamp.tile([128, N], BF16, tag="tmp", bufs=1,
                                   name=f"tmp_{kt}")
                t1 = streamp.tile([128, N], BF16, tag="t1", bufs=1,
                                  name=f"t1_{kt}")
                t3 = streamp.tile([128, N], BF16, tag="t3", bufs=1,
                                  name=f"t3_{kt}")
                nc.scalar.activation(t1[:], ats[1][:], ACTF.Copy,
                                     scale=s_bc[:, 11:12])
                nc.scalar.activation(t3[:], ats[3][:], ACTF.Copy,
                                     scale=s_bc[:, 13:14])
                nc.vector.tensor_scalar_mul(acc0[:], ats[0][:], s_bc[:, 10:11])
                nc.vector.tensor_add(acc1[:], t1[:], acc0[:])
                nc.vector.tensor_scalar_mul(tmp[:], ats[2][:], s_bc[:, 12:13])
                nc.vector.tensor_add(acc0[:], tmp[:], acc1[:])
                nc.vector.tensor_add(acc1[:], t3[:], acc0[:])
                nc.vector.tensor_scalar_mul(tmp[:], ats[4][:], s_bc[:, 14:15])
                nc.vector.tensor_add(a1_sb[:, N * kt:N * (kt + 1)], tmp[:],
                                     acc1[:])
                # PE: a_shT tile from fp8 transposed-rows stream
                psa = psp.tile([128, 512], F32, tag="ps", name=f"aps{kt}")
                for e in range(E):
                    art = streamp.tile([128, R], FP8, tag="art", bufs=10,
                                       name=f"art{kt}_{e}")
                    nc.sync.dma_start(art[:],
                                      a_rowst[e, 128 * kt:128 * (kt + 1), :])
                    nc.tensor.matmul(psa[:, :R],
                                     eyes[:, 128 * e:128 * (e + 1)], art[:],
                                     start=(e == 0), stop=(e == E - 1))
                nc.scalar.activation(a_shT[:, R * kt:R * (kt + 1)], psa[:, :R],
                                     ACTF.Copy, accum_out=csa[:, kt:kt + 1])
                nc.vector.tensor_copy(csa_bf[:, kt:kt + 1], csa[:, kt:kt + 1])
                nc.vector.tensor_copy(
                    csa_rep[:, 128 * kt:128 * (kt + 1)],
                    csa_bf[:, kt:kt + 1].broadcast_to([128, 128]))
                for nc4 in range(4):
                    nc.tensor.matmul(
                        degps[nc4][:],
                        csa_rep[:, 128 * kt:128 * (kt + 1)],
                        b_sb[:, N * kt + 512 * nc4:N * kt + 512 * (nc4 + 1)],
                        start=(kt == 0), stop=(kt == KT - 1))


            # ---- deg1 = csa . b  (column sums of H1 over local rows, incl
            # diag: a ~5e-4 relative effect on deg, far inside tolerance).
            # Fires the AllReduce before mm1' so the rendezvous hides there.
            degrow = smallp.tile([1, N], F32, tag="degrow")
            for nc4 in range(4):
                nc.vector.tensor_copy(degrow[0:1, 512 * nc4:512 * (nc4 + 1)],
                                      degps[nc4][0:1, :])
            nc.sync.dma_start(deg_in[:], degrow[:])
            nc.gpsimd.collective_compute(
                "AllReduce", ALU.add, replica_groups=GROUPS,
                ins=[deg_in[:].opt()], outs=[deg_out[:].opt()])
            degf = smallp.tile([128, KT], F32, tag="degf")
            nc.sync.dma_start(
                degf[:],
                deg_out[0:1, :].rearrange("o (k p) -> p (o k)", p=128))
            dege = smallp.tile([128, KT], F32, tag="dege")
            nc.vector.tensor_scalar_add(dege[:], degf[:], float(EPS))
            rcp = smallp.tile([128, KT], F32, tag="rcp")
            nc.vector.reciprocal(rcp[:], dege[:])
            dinv = smallp.tile([128, KT], F32, tag="dinv")
            nc.vector.tensor_scalar_mul(dinv[:], rcp[:], float(S))

            # ---- mm1': H1T = b.T @ a_shT  (fp8 DoubleRow, K=256/instr)
            h1tm = bigp.tile([128, KT * R], BF16, tag="slot_h1tm")
            bv = b_sb[:].rearrange("p (k n) -> p k n", k=KT)
            av = a_shT[:].rearrange("p (k m) -> p k m", k=KT)
            for half in range(2):
                nts = range(8 * half, 8 * half + 8)
                pss = {}
                for nt in nts:
                    pss[nt] = psp.tile([128, 512], F32, tag="ps", name=f"h1t_ps{nt}")
                for kp in range(KT // 2):
                    kt = 2 * kp
                    for nt in nts:
                        nc.tensor.matmul(
                            pss[nt][:, :R],
                            bv[:, kt:kt + 2, 128 * nt:128 * (nt + 1)],
                            av[:, kt:kt + 2, :],
                            start=(kp == 0), stop=(kp == KT // 2 - 1),
                            perf_mode=DR)
                for nt in nts:
                    nc.scalar.activation(h1tm[:, R * nt:R * (nt + 1)],
                                         pss[nt][:, :R], ACTF.Copy)

            # lhsT2 = (H1T masked) * S/(deg+eps)  -> fp8  (one fused op/tile)
            lhsT2 = bigp.tile([128, KT * R], FP8, tag="slot_lhsT2")
            for kt in range(KT):
                m1 = streamp.tile([128, R], BF16, tag="m1", bufs=4,
                                  name=f"m1_{kt}")
                nc.sync.dma_start(m1[:], mask1_in[128 * kt:128 * (kt + 1), :])
                nc.vector.scalar_tensor_tensor(
                    lhsT2[:, R * kt:R * (kt + 1)],
                    h1tm[:, R * kt:R * (kt + 1)], dinv[:, kt:kt + 1], m1[:],
                    op0=ALU.mult, op1=ALU.mult)

            # ---- mm2: S*H2 = lhsT2.T @ a1  (fp8 DoubleRow), diag := S
            h2 = bigp.tile([128, RT * N], BF16, tag="slot_h2")
            lv = lhsT2[:].rearrange("p (k m) -> p k m", k=KT)
            a1v = a1_sb[:].rearrange("p (k n) -> p k n", k=KT)
            ones_t = constp.tile([128, 1024], BF16)
            nc.vector.memset(ones_t[:], float(S))
            for half in range(2):
                ps2 = {}
                for mt in range(RT):
                    for nb in range(2):
                        ps2[(mt, nb)] = psp.tile([128, 512], F32, tag="ps",
                                                 name=f"h2_ps{mt}_{nb}")
                for kp in range(KT // 2):
                    kt = 2 * kp
                    noff = 1024 * half
                    for mt in range(RT):
                        lt = lv[:, kt:kt + 2, 128 * mt:128 * (mt + 1)]
                        for nb in range(2):
                            nc.tensor.matmul(
                                ps2[(mt, nb)],
                                lt,
                                a1v[:, kt:kt + 2,
                                    noff + 512 * nb:noff + 512 * (nb + 1)],
                                start=(kp == 0), stop=(kp == KT // 2 - 1),
                                perf_mode=DR)
                for mt in range(RT):
                    for nb in range(2):
                        dst = h2[:, N * mt + 1024 * half + 512 * nb:
                                 N * mt + 1024 * half + 512 * (nb + 1)]
                        if nb == 0:
                            nc.vector.tensor_copy(dst, ps2[(mt, nb)][:])
                        else:
                            nc.scalar.activation(dst, ps2[(mt, nb)][:], ACTF.Copy)
                for mt in range(RT):
                    m2 = streamp.tile([128, 1024], U8, tag="m2", bufs=2,
                                      name=f"m2_{mt}_{half}")
                    nc.sync.dma_start(
                        m2[:], mask2_in[128 * mt:128 * (mt + 1),
                                        1024 * half:1024 * (half + 1)])
                    nc.vector.copy_predicated(
                        h2[:, N * mt + 1024 * half:N * mt + 1024 * (half + 1)],
                        m2[:], ones_t[:])

            # ---- Xw = X @ W with ones column  [R, 65] bf16
            xw = constp.tile([128, RT * (WOUT + 1)], BF16)
            nc.vector.memset(xw[:], 1.0)
            xt_sb = constp.tile([128, 2 * R], BF16)
            nc.sync.dma_start(
                xt_sb[:].rearrange("p (f r) -> p f r", f=2),
                xt_loc[:].rearrange("(f p) r -> p f r", p=128))
            wg_sb = constp.tile([128, 2 * WOUT], BF16)
            nc.sync.dma_start(
                wg_sb[:].rearrange("p (f r) -> p f r", f=2),
                w_gcn[:].rearrange("(f p) r -> p f r", p=128))
            for jt in range(RT):
                psx = psp.tile([128, 512], F32, tag="ps", name=f"xw_ps{jt}")
                for ft in range(2):
                    nc.tensor.matmul(
                        psx[:, :WOUT],
                        xt_sb[:, R * ft + 128 * jt:R * ft + 128 * (jt + 1)],
                        wg_sb[:, WOUT * ft:WOUT * (ft + 1)],
                        start=(ft == 0), stop=(ft == 1))
                nc.vector.tensor_copy(
                    xw[:, (WOUT + 1) * jt:(WOUT + 1) * jt + WOUT], psx[:, :WOUT])

            # ---- mm3: P = (S H2').T @ Xw'  [N, 65] partials.
            # Two interleaved ReduceScatters so the first rendezvous overlaps
            # the second half of mm3 and the finalize of set A overlaps RS_B.
            # Set A = row-tiles it%4 in {0,1} -> out rows [512q, 512q+256);
            # set B = it%4 in {2,3} -> rows [512q+256, 512q+512).
            its_A = [it for it in range(KT) if it % 4 < 2]
            its_B = [it for it in range(KT) if it % 4 >= 2]
            for sidx, (its, pin) in enumerate([(its_A, p_inA), (its_B, p_inB)]):
                for it in its:
                    psp3 = psp.tile([128, 512], F32, tag="ps", name=f"p_ps{it}")
                    for jt in range(RT):
                        nc.tensor.matmul(
                            psp3[:, :WOUT + 1],
                            h2[:, N * jt + 128 * it:N * jt + 128 * (it + 1)],
                            xw[:, (WOUT + 1) * jt:(WOUT + 1) * (jt + 1)],
                            start=(jt == 0), stop=(jt == RT - 1))
                    row = 256 * (it // 4) + 128 * (it % 4 - 2 * sidx)
                    pt = streamp.tile([128, WOUT + 1], F32, tag="pt", bufs=4,
                                      name=f"pt{it}")
                    nc.vector.tensor_copy(pt[:], psp3[:, :WOUT + 1])
                    nc.sync.dma_start(pin[row:row + 128, :], pt[:])
                nc.gpsimd.collective_compute(
                    "ReduceScatter", ALU.add, replica_groups=GROUPS,
                    ins=[pin[:].opt()],
                    outs=[(p_outA if sidx == 0 else p_outB)[:].opt()])

            # ---- finalize: out = relu(P' / (deg2' + S*eps))   (S cancels)
            for sidx, pout in enumerate([p_outA, p_outB]):
                pf = smallp.tile([128, 2 * (WOUT + 1)], F32, tag=f"pf{sidx}",
                                 name=f"pf{sidx}")
                nc.sync.dma_start(
                    pf[:].rearrange("p (t f) -> p t f", t=2),
                    pout[:].rearrange("(t p) f -> p t f", p=128))
                of = smallp.tile([128, 2 * WOUT], F32, tag=f"of{sidx}",
                                 name=f"of{sidx}")
                for t in range(2):
                    d2 = smallp.tile([128, 1], F32, tag=f"d2_{sidx}_{t}",
                                     name=f"d2_{sidx}_{t}")
                    nc.vector.tensor_scalar_add(
                        d2[:], pf[:, (WOUT + 1) * t + WOUT:(WOUT + 1) * (t + 1)],
                        float(S * EPS))
                    r2 = smallp.tile([128, 1], F32, tag=f"r2_{sidx}_{t}",
                                     name=f"r2_{sidx}_{t}")
                    nc.vector.reciprocal(r2[:], d2[:])
                    nc.vector.tensor_scalar(
                        of[:, WOUT * t:WOUT * (t + 1)],
                        pf[:, (WOUT + 1) * t:(WOUT + 1) * t + WOUT],
                        r2[:], 0.0, op0=ALU.mult, op1=ALU.max)
                nc.sync.dma_start(
                    out_loc[256 * sidx:256 * (sidx + 1), :]
                        .rearrange("(t p) f -> p t f", p=128),
                    of[:].rearrange("p (t f) -> p t f", t=2))

    nc.compile()
    return nc


_NC_CACHE = []


def get_nc():
    if not _NC_CACHE:
        _NC_CACHE.append(_build_nc())
    return _NC_CACHE[0]


def make_in_maps(A, X, conv_w_l0_1, conv_w_l0_2, conv_w_l1, gcn_weight):
    A = np.asarray(A, np.float32)
    X = np.asarray(X, np.float32)
    w1 = np.asarray(conv_w_l0_1, np.float32)[:, :, 0, 0]
    w2 = np.asarray(conv_w_l0_2, np.float32)[:, :, 0, 0]
    w3 = np.asarray(conv_w_l1, np.float32)[:, :, 0, 0]
    W = np.asarray(gcn_weight, np.float32)

    a_bf = A.astype(_BF)
    eye = np.eye(128, dtype=_BF)
    wg = W.astype(_BF)

    in_maps = []
    for core in range(NCORES):
        c, q = core // 4, core % 4
        rows = slice(R * q, R * (q + 1))
        a_rowst = np.ascontiguousarray(
            A[:, rows, :].transpose(0, 2, 1)).astype(_F8)
        wrow = np.concatenate([w1[c], w2[c], w3[c]]).reshape(1, 15)
        wbc = np.ascontiguousarray(np.tile(wrow, (128, 1))).astype(np.float32)
        m1 = np.ones((N, R), dtype=_BF)
        g = np.arange(R)
        m1[R * q + g, g] = 0.0
        m2 = np.zeros((R, N), dtype=np.uint8)
        m2[g, R * q + g] = 1
        xt = np.ascontiguousarray(X[rows].T).astype(_BF)
        in_maps.append({
            "a_full": a_bf,
            "a_rowst": a_rowst,
            "wbc": wbc,
            "eye_in": eye,
            "mask1": m1,
            "mask2": m2,
            "xt_loc": xt,
            "w_gcn": wg,
        })
    return in_maps


def assemble(results):
    out = np.empty((N, C * WOUT), np.float32)
    for core in range(NCORES):
        c, q = core // 4, core % 4
        out[R * q:R * (q + 1), WOUT * c:WOUT * (c + 1)] = \
            results[core]["out_loc"]
    return out


def kernel(A, X, conv_w_l0_1, conv_w_l0_2, conv_w_l1, gcn_weight, **run_kwargs):
    nc = get_nc()
    in_maps = make_in_maps(A, X, conv_w_l0_1, conv_w_l0_2, conv_w_l1,
                           gcn_weight)
    res = run_bass_kernel_spmd(nc, in_maps, core_ids=list(range(NCORES)),
                               **run_kwargs)
    out = assemble(res.results)
    if run_kwargs:
        kernel.last_results = res
    return out
